# revision 58
# baseline (speedup 1.0000x reference)
"""MiniBatchDiscrimination kernel for 8 Trainium2 NeuronCores.

Problem:
  x [256, 1024] f32, T [1024, 128, 16] f32
  M = einsum('na,abc->nbc', x, T)                      [N=256, B=128, C=16]
  D[k,j,b] = sum_c |M[k,b,c] - M[j,b,c]|
  Cmat = exp(-D); S = sum_j Cmat
  out = S - Cmat[:, N-1, :]; out[0] = S[0]-Cmat[0,0]; out[N-1] = S[N-1]-Cmat[N-1,N-1]

Sharding: data-parallel over B (each core owns 16 of the 128 b-channels).
The pairwise distance is independent per b, so there is no communication.

Per-core dataflow (abs decomposed as |d| = 2*relu(d) - d, since abs_max is
not a valid TRN2 DVE ALU op but relu (sub,max,0) is a single 2x-mode op;
the linear term sum_c d = R[b,j] - R[b,k] is folded in by PE and the exp
bias). Everything streams in fp16 except the f32 PSUM accumulations:
  PE   : MT[bc, n] = (x @ T_loc)^T via 16 accumulating matmuls (a-chunks)
  PE   : R[b, j] = sum_c M[j, b, c]  (pattern matmul)
  DVE  : per k: relu(MT[:, j] - MT[:, k]) via tensor_scalar(sub, max, 0),
         4x perf mode; ~1 in 11 k's runs on ScalarE activation(Relu)
         instead, emitted one group ahead so a blocked exp can't delay it
  PE   : c-reduction: 2*pattern^T @ relu-tile -> 2P, 4 k's per [128, 256]
         PSUM bank via col-group tile_position (16-row slices at 32g); one
         fold matmul per bank adds -R[b, j]
  ScE  : exp(-psD + bias), bias = -R[b,k] per partition; accum_out emits
         the row sums S (the j-reduction) for free
  Pool : extract Cmat[:, 255] columns (and Cmat[0,0])
  DVE  : final out = S - C255 (+ k=0 self fix), in two halves to overlap
         the output DMAs with the second half of the main loop
"""

import os
import sys

import numpy as np

for _p in ("/opt/trn_rl_repo", os.path.expanduser("~/.axon_site/_ro/trn_rl_repo")):
    if os.path.isdir(_p) and _p not in sys.path:
        sys.path.insert(0, _p)
        break

import concourse.bass as bass
import concourse.tile as tile
from concourse import bacc, mybir
from concourse.bass_utils import run_bass_kernel_spmd

A, B, C, N = 1024, 128, 16, 256
NCORES = 8
BL = B // NCORES          # 16 b-channels per core
BC = BL * C               # 256 (b, c) pairs per core
NGROUPS = N // 8          # 32 groups of 8 k-values
F32 = mybir.dt.float32
ALU = mybir.AluOpType
AF = mybir.ActivationFunctionType

DT_STREAM = mybir.dt.float16  # dtype of the absdiff stream path (F32 or float16)
F32R = mybir.dt.float32r


def on_act(k: int) -> bool:
    """k's whose relu-diff runs on ScalarE (load balancing vs DVE)."""
    return k % 8 == 3 and k % 32 != 27


_cache = {}


def _patterns():
    # patA maps bc-block0 partitions (b = p//16 in 0..7) to out row b;
    # patB maps bc-block1 partitions to out rows 8 + p//16.
    patA = np.zeros((128, 16), np.float32)
    patB = np.zeros((128, 16), np.float32)
    for p in range(128):
        patA[p, p // 16] = 1.0
        patB[p, 8 + p // 16] = 1.0
    # fold weight: out[32g + b, :] += rhs[b, :]
    foldW = np.zeros((16, 128), np.float32)
    for m in range(128):
        if m % 32 < 16:
            foldW[m % 32, m] = 1.0
    return patA, patB, foldW


def build_program(dbg: bool = False):
    nc = bacc.Bacc(
        "TRN2", target_bir_lowering=False, debug=False, enable_asserts=True
    )

    xT_d = nc.dram_tensor("xT", [A, N], DT_STREAM, kind="ExternalInput")
    tl_d = nc.dram_tensor("Tl", [A, BC], DT_STREAM, kind="ExternalInput")
    out_d = nc.dram_tensor("out", [N, BL], F32, kind="ExternalOutput")
    if dbg:
        mt_o = nc.dram_tensor("mt_o", [2, 128, N], F32, kind="ExternalOutput")
        negR_o = nc.dram_tensor("negR_o", [16, N], F32, kind="ExternalOutput")
        psD_o = nc.dram_tensor("psD_o", [2, 128, 512], F32, kind="ExternalOutput")
        sall_o = nc.dram_tensor("sall_o", [128, 64], F32, kind="ExternalOutput")
        c255_o = nc.dram_tensor("c255_o", [128, 64], F32, kind="ExternalOutput")

    np_dt = np.float32 if DT_STREAM == F32 else np.float16
    patA_np, patB_np, foldW_np = _patterns()
    pats_np = np.concatenate(
        [patA_np, patB_np, 2 * patA_np, 2 * patB_np], axis=1)  # [128, 64]
    pats_d = nc.inline_tensor(pats_np.astype(np_dt), name="pats")
    foldW_d = nc.inline_tensor(foldW_np.astype(np_dt), name="foldW")

    xT_v = xT_d[:].rearrange("(a p) n -> p a n", p=128)
    tl_v = tl_d[:].rearrange("(a p) m -> p a m", p=128)

    with tile.TileContext(nc) as tc:
        with (
            tc.tile_pool(name="persist", bufs=1) as pp,
            tc.tile_pool(name="ad", bufs=24) as adp,
            tc.tile_pool(name="cm", bufs=8) as cmp_,
            tc.tile_pool(name="psum_d", bufs=6, space="PSUM") as pd,
        ):
            # ---- load inputs (split across both HWDGE rings: SP and ACT) ----
            xbig = pp.tile([128, 8 * N], DT_STREAM, tag="xbig")
            tbig = pp.tile([128, 8 * BC], DT_STREAM, tag="tbig")
            xbv = xbig[:].rearrange("p (a n) -> p a n", a=8)
            tbv = tbig[:].rearrange("p (a m) -> p a m", a=8)
            # first a-chunk alone so the GEMM can start ~1.5us in
            nc.sync.dma_start(xbv[:, 0:1], xT_v[:, 0:1])
            nc.scalar.dma_start(tbv[:, 0:1], tl_v[:, 0:1])
            nc.sync.dma_start(xbv[:, 1:4], xT_v[:, 1:4])
            nc.scalar.dma_start(tbv[:, 1:4], tl_v[:, 1:4])
            nc.scalar.dma_start(xbv[:, 4:8], xT_v[:, 4:8])
            nc.sync.dma_start(tbv[:, 4:8], tl_v[:, 4:8])
            xts = [xbig[:, a * N:(a + 1) * N] for a in range(8)]
            tls = [tbig[:, a * BC:(a + 1) * BC] for a in range(8)]

            pats_t = pp.tile([128, 64], DT_STREAM, tag="pats")
            nc.sync.dma_start(pats_t[:], pats_d[:])
            pats = {
                "patA1": pats_t[:, 0:16], "patB1": pats_t[:, 16:32],
                "patA2": pats_t[:, 32:48], "patB2": pats_t[:, 48:64],
            }
            foldW_t = pp.tile([16, 128], DT_STREAM, tag="foldW")
            nc.scalar.dma_start(foldW_t[:], foldW_d[:])

            # ---- GEMM: MT[bc, n] = sum_a Tl[a, bc] * x[n, a] ----
            pmt_ctx = tc.tile_pool(name="psum_mt", bufs=2, space="PSUM")
            pmt = pmt_ctx.__enter__()
            MT = []        # stream dtype (input of absdiff)
            MTs = []       # f32 scalar source for tensor_scalar scalar1
            negMT = []     # f32, bias source for ScalarE Abs
            for blk in range(2):
                ps = pmt.tile([128, N], F32, tag="psmt")
                for a in range(8):
                    nc.tensor.matmul(
                        ps[:],
                        tls[a][:, blk * 128:(blk + 1) * 128],
                        xts[a],
                        start=(a == 0),
                        stop=(a == 7),
                    )
                mt_t = pp.tile([128, N], DT_STREAM, tag=f"mt{blk}")
                nc.scalar.copy(mt_t[:], ps[:])
                if DT_STREAM == F32:
                    mts_t = mt_t
                else:
                    mts_t = pp.tile([128, N], F32, tag=f"mts{blk}")
                    nc.vector.tensor_copy(mts_t[:], mt_t[:])
                nmt_t = pp.tile([128, N], F32, tag=f"nmt{blk}")
                nc.vector.tensor_scalar(
                    out=nmt_t[:], in0=mts_t[:], scalar1=-1.0, scalar2=None,
                    op0=ALU.mult,
                )
                MT.append(mt_t)
                MTs.append(mts_t)
                negMT.append(nmt_t)

            # ---- R[b, j] = sum_c M[j, b, c]; negR = -R ----
            psR = pmt.tile([16, N], F32, tag="psmt")
            nc.tensor.matmul(psR[:], pats["patA1"], MT[0][:],
                             start=True, stop=False)
            nc.tensor.matmul(psR[:], pats["patB1"], MT[1][:],
                             start=False, stop=True)
            # negR in stream dtype: the fold matmul adds exactly these values,
            # and the exp bias below must cancel them bit-exactly on j == k.
            negR = pp.tile([16, N], DT_STREAM, tag="negR")
            nc.scalar.mul(negR[:], psR[:], -1.0)
            pmt_ctx.__exit__(None, None, None)

            # negRbias: [128, 64]; col 2G+h rows 32g+b = -R[b, 8G+4h+g]
            negRb = pp.tile([128, 2 * NGROUPS], DT_STREAM, tag="negRb")
            nc.vector.memset(negRb[:], 0.0)
            for g in range(4):
                src = negR[:].rearrange("b (q g) -> b q g", g=4)[:, :, g]
                nc.sync.dma_start(negRb[32 * g:32 * g + 16, :], src)

            # ---- persistent result tiles ----
            S_all = pp.tile([128, 2 * NGROUPS], F32, tag="S_all")
            C255 = pp.tile([128, 2 * NGROUPS], F32, tag="C255")
            C00 = pp.tile([16, 1], F32, tag="C00")
            R_all = pp.tile([128, 2 * NGROUPS], F32, tag="R_all")
            # out row k = 8G+4h+g, col b  <-  R_all[32g+b, 2G+h]
            dstv = out_d[:].rearrange("(G h g) b -> g b G h", G=NGROUPS, h=2, g=4)

            def make_act_ads(GG, store):
                for h in range(2):
                    for g in range(4):
                        k = 8 * GG + 4 * h + g
                        if not on_act(k):
                            continue
                        pair = []
                        for blk in range(2):
                            ad_t = adp.tile([128, N], DT_STREAM, tag="adact")
                            nc.scalar.activation(
                                ad_t[:], MT[blk][:], AF.Relu,
                                bias=negMT[blk][:, k:k + 1], scale=1.0,
                            )
                            pair.append(ad_t)
                        store[k] = pair

            def finalize_half(lo, hi):
                # out[k] = S[k] - Cmat[k, 255] for cols [lo, hi)
                nc.vector.tensor_tensor(
                    out=R_all[:, lo:hi], in0=S_all[:, lo:hi],
                    in1=C255[:, lo:hi], op=ALU.subtract,
                )
                if lo == 0:
                    # out[0] = S[0] - Cmat[0, 0]
                    nc.vector.tensor_tensor(
                        out=R_all[0:16, 0:1], in0=S_all[0:16, 0:1],
                        in1=C00[:], op=ALU.subtract,
                    )
                # (k=255 -> col 63: its C255 value IS Cmat[255,255]; no fix)
                for g in range(4):
                    srcv = R_all[32 * g:32 * g + 16, lo:hi].rearrange(
                        "b (G h) -> b G h", h=2)
                    nc.sync.dma_start(dstv[g][:, lo // 2:hi // 2, :], srcv)

            # ---- main loop ----
            act_ads = {}
            make_act_ads(0, act_ads)
            for G in range(NGROUPS):
                if G + 1 < NGROUPS:
                    # ScalarE relu-diffs for the NEXT group, queued ahead of
                    # this group's exps so a blocked exp can't delay them
                    make_act_ads(G + 1, act_ads)
                if G == NGROUPS // 2:
                    finalize_half(0, NGROUPS)
                for h in range(2):
                    # own PSUM bank per half: the exp (ACT read) must not
                    # share a bank with the next half's PE writes, or Tile
                    # serializes them (bank-overlap tracking).
                    psDh = pd.tile([128, N], F32, tag="psD")
                    for g in range(4):
                        k = 8 * G + 4 * h + g
                        if on_act(k):
                            ads = act_ads.pop(k)
                        else:
                            ads = []
                            for blk in range(2):
                                ad_t = adp.tile([128, N], DT_STREAM, tag="ad")
                                nc.vector.tensor_scalar(
                                    out=ad_t[:], in0=MT[blk][:],
                                    scalar1=MTs[blk][:, k:k + 1], scalar2=0.0,
                                    op0=ALU.subtract, op1=ALU.max,
                                )
                                ads.append(ad_t)
                        outsl = psDh[32 * g:32 * g + 16, :]
                        nc.tensor.matmul(
                            outsl, pats["patA2"], ads[0][:],
                            start=True, stop=False, tile_position=(0, 32 * g),
                        )
                        nc.tensor.matmul(
                            outsl, pats["patB2"], ads[1][:],
                            start=False, stop=False,
                            tile_position=(0, 32 * g),
                        )
                    # psDh += -R[b, j] broadcast over the four 32-row groups
                    nc.tensor.matmul(
                        psDh[:], foldW_t[:], negR[:],
                        start=False, stop=True, skip_group_check=True,
                    )
                    if dbg and G in (0, 4):
                        dcp = cmp_.tile([128, N], F32, tag="dcp")
                        nc.vector.tensor_copy(dcp[:], psDh[:])
                        nc.sync.dma_start(
                            psD_o[:][0 if G == 0 else 1][:, N * h:N * (h + 1)],
                            dcp[:])
                    col = 2 * G + h
                    cm_t = cmp_.tile([128, N], F32, tag="cm")
                    nc.scalar.activation(
                        cm_t[:], psDh[:], AF.Exp,
                        bias=negRb[:, col:col + 1],
                        scale=-1.0,
                        accum_out=S_all[:, col:col + 1],
                    )
                    nc.gpsimd.tensor_copy(C255[:, col:col + 1], cm_t[:, 255:256])
                    if G == 0 and h == 0:
                        # Cmat[0, 0, :] lives at rows 0..15, j-col 0 (k=0 is g=0)
                        nc.gpsimd.tensor_copy(C00[:], cm_t[0:16, 0:1])

            finalize_half(NGROUPS, 2 * NGROUPS)

            if dbg:
                for blk in range(2):
                    nc.sync.dma_start(mt_o[:][blk], MTs[blk][:])
                nc.sync.dma_start(negR_o[:], negR[:])
                nc.sync.dma_start(sall_o[:], S_all[:])
                nc.sync.dma_start(c255_o[:], C255[:])

    nc.compile()
    return nc


def kernel(x: np.ndarray, T: np.ndarray) -> np.ndarray:
    if "nc" not in _cache:
        _cache["nc"] = build_program()
    nc = _cache["nc"]

    np_dt = np.float32 if DT_STREAM == F32 else np.float16
    x = np.ascontiguousarray(x, dtype=np.float32)
    T = np.ascontiguousarray(T, dtype=np.float32)
    xT = np.ascontiguousarray(x.T.astype(np_dt))         # [A, N]

    in_maps = []
    for c in range(NCORES):
        tl = np.ascontiguousarray(
            T[:, c * BL:(c + 1) * BL, :].reshape(A, BC).astype(np_dt))
        in_maps.append({"xT": xT, "Tl": tl})

    res = run_bass_kernel_spmd(nc, in_maps, list(range(NCORES)))
    outs = [res.results[c]["out"] for c in range(NCORES)]
    return np.concatenate(outs, axis=1)                  # [N, B]


if __name__ == "__main__":
    rng = np.random.default_rng(0)
    x = rng.standard_normal((N, A)).astype(np.float32)
    T = rng.random((A, B, C), dtype=np.float32)
    out = kernel(x, T)
    print(out.shape, out.dtype, out[:3, :3])
